# revision 22
# baseline (speedup 1.0000x reference)
"""Multi-head causal attention (B=4, T=2048, 16 heads x 64) on 8 trn2 NeuronCores.

Sharding: tensor-parallel over heads (2 heads/core) for QKV projection +
attention; one AllToAll reshard per half-batch (head-sharded -> token-sharded),
fired as each half-batch finishes so only the last one is exposed; the output
projection for each half-batch's tokens is interleaved one batch behind.

The PE streams ~1 row/cycle at ~1.2 GHz regardless of operand dtype, so PE
time is cut by cutting ROWS: QKV projection and A'V run in fp8e4m3 DoubleRow
mode (two contraction slabs per instruction, 0.5 cycles/row); scores stay
bf16 (K=64, both heads packed via tile_position); the output projection is
bf16. fp8 quantization errors average out under attention. Weights are
host-scaled by 64 so fp8e4m3 sees mid-range values; the 1/64 is folded into
the PSUM->SBUF copies.

Per-core dataflow:
  - x.T host-packed to the exact SBUF layout [128, chunk, T] in fp8 (one
    16 KB descriptor per partition - the naive rearrange DMA was
    descriptor-generation-bound at ~19 us); one tile per batch.
  - QKV DoubleRow: lhsT = w[128, 2 slabs, 128], rhs = xt[:, 2c:2c+2, tq tile]
    -> Q.T, K.T [128=2*64 headdim, T] bf16 (x 1/64); V.T fp8 (x 1/64); V.T is
    PE-transposed into V_aug [ks, chunk, 256] fp8: per chunk
    [V_h0 | pad=1, V_h1 | pad=1] with col 64/192 the softmax-denominator
    ones column (cols 65..127 also 1.0: PSUM rows 65..127 are ignored;
    a 128-wide stationary streams 1 c/row where 65-wide streamed 1.5).
  - Scores transposed, S.T[ks,tq] = K @ Q.T, diagonal chunks trimmed to the
    causal region; ks-chunk PAIRS share one [128,1024] PSUM tile; Exp on the
    pair in one ACT op (diagonal pairs: two trimmed Exps + a zeroed
    above-diagonal block for the DoubleRow union range) -> E fp8; one
    128x128 triangular mask multiply per diagonal chunk. Softmax without
    max-subtraction (scores in [-0.6, 0.6]).
  - A'V DoubleRow per chunk-pair: rhs = E [128, 2, tq], lhsT = V_aug pair
    -> O.T|denom [128, tq] accumulated. Score matmuls for pair k+1 are
    emitted before A'V of pair k so the PE never waits on the Exp.
  - normalize: copy O.T out of PSUM (frees the bank); denom row reshaped
    [1,512]->[4,128] by SBUF DMA so the DVE reciprocal runs 4 lanes
    (0.85us vs 3.34); gpsimd partition_broadcast x4; DVE mul -> bf16.
  - output projection (bf16) over the 8 gathered head-chunks; bias add.

One AllToAll per half-batch (8 total; all but the last hidden under
compute), each moving bf16 [8 shards, 128 hd, 128 tok]: for half-batch
(b, jj), shard (j-2jj)*4+q holds tokens b*2048 + j*512 + q*128 + [0,128);
rank r owns (j = 2jj + r//4, q = r%4) of every half-batch = 8 x 128 rows.
"""

import numpy as np
import ml_dtypes

import concourse.bacc as bacc
import concourse.tile as tile
from concourse import mybir
from concourse.bass_utils import run_bass_kernel_spmd

NCORES = 8
B, T, C, H, D = 4, 2048, 1024, 16, 64
TQ = 512          # moving-dim tile for scores / A'V
NKC = T // 128    # ks 128-chunks per batch (16)
NJ = T // TQ      # tq tiles per batch (4)
NCC = C // 128    # contraction chunks for projections (8)
WSCALE = 64.0     # host-side qkv weight scale for fp8 range

f32 = mybir.dt.float32
bf16 = mybir.dt.bfloat16
fp8 = mybir.dt.float8e4
AF = mybir.ActivationFunctionType
DR = mybir.MatmulPerfMode.DoubleRow
BF16 = ml_dtypes.bfloat16
FP8 = ml_dtypes.float8_e4m3


def build_bass():
    nc = bacc.Bacc(None, num_devices=NCORES)

    # x.T pre-packed to SBUF layout: [b][partition][chunk][t], fp8
    xT = nc.dram_tensor("xT", [B, 128, NCC, T], fp8, kind="ExternalInput")
    # bf16 x.T for the V chain, packed per half-batch: [b][half][p][chunk][t]
    xT16 = nc.dram_tensor("xT16", [B, 2, 128, NCC, 2 * TQ], bf16,
                          kind="ExternalInput")
    # per-core q,k weights (x WSCALE), host-prearranged: [row, 2(q,k), chunk, 2D]
    w_in = nc.dram_tensor("w", [128, 2, NCC, 2 * D], fp8, kind="ExternalInput")
    # per-core v weights, bf16 unscaled
    wv_in = nc.dram_tensor("wv", [128, NCC, 2 * D], bf16, kind="ExternalInput")
    # proj_w.T host-prearranged: [row-in-chunk, chunk, C]
    pw_in = nc.dram_tensor("pw", [128, NCC, C], bf16, kind="ExternalInput")
    bias_in = nc.dram_tensor("biasb", [128, C], f32, kind="ExternalInput")
    y_out = nc.dram_tensor("y", [B * T // NCORES, C], f32, kind="ExternalOutput")

    ident_np = np.eye(128, dtype=np.float32).astype(BF16)
    tri_np = (np.arange(128)[None, :] >= np.arange(128)[:, None]).astype(BF16)
    ident_dram = nc.inline_tensor(ident_np, name="ident")
    tri_dram = nc.inline_tensor(tri_np, name="trimask")

    with tile.TileContext(nc, num_cores=NCORES) as tc, nc.allow_low_precision(
        reason="fp8/bf16 kernel; tolerance 2e-2"
    ):
        with (
            tc.tile_pool(name="dram", bufs=1, space="DRAM") as dpool,
            tc.tile_pool(name="consts", bufs=1) as consts,
            tc.tile_pool(name="xt", bufs=2) as xt_pool,
            tc.tile_pool(name="qt", bufs=2) as qt_pool,
            tc.tile_pool(name="kt", bufs=2) as kt_pool,
            tc.tile_pool(name="vt", bufs=1) as vt_pool,
            tc.tile_pool(name="vaug", bufs=2) as vaug_pool,
            tc.tile_pool(name="e", bufs=4) as e_pool,
            tc.tile_pool(name="onorm", bufs=4) as onorm_pool,
            tc.tile_pool(name="small", bufs=2) as small_pool,
            tc.tile_pool(name="proj", bufs=2) as proj_pool,
            tc.tile_pool(name="ytile", bufs=2) as y_pool,
            tc.tile_pool(name="ps_a", bufs=2, space="PSUM") as ps_a,
            tc.tile_pool(name="ps_s", bufs=2, space="PSUM") as ps_s,
            tc.tile_pool(name="ps_o", bufs=1, space="PSUM") as ps_o,
        ):
            sends = [
                dpool.tile([NCORES, 2 * D, 128], bf16, tag="send", name=f"send{hb}")
                for hb in range(2 * B)
            ]
            recvs = [
                dpool.tile([NCORES, 2 * D, 128], bf16, tag="recv", name=f"recv{hb}")
                for hb in range(2 * B)
            ]

            w_sb = consts.tile([128, 2, NCC, 2 * D], fp8)
            nc.gpsimd.dma_start(out=w_sb[:], in_=w_in[:])
            wv_sb = consts.tile([128, NCC, 2 * D], bf16)
            nc.gpsimd.dma_start(out=wv_sb[:], in_=wv_in[:])
            ident_sb = consts.tile([128, 128], bf16)
            nc.gpsimd.dma_start(out=ident_sb[:], in_=ident_dram[:])
            tri_sb = consts.tile([128, 128], bf16)
            nc.gpsimd.dma_start(out=tri_sb[:], in_=tri_dram[:])
            pw_sb = consts.tile([128, NCC, C], bf16)
            bias_sb = consts.tile([128, C], f32)

            def load_proj_consts():
                nc.gpsimd.dma_start(out=pw_sb[:], in_=pw_in[:])
                nc.gpsimd.dma_start(out=bias_sb[:], in_=bias_in[:])

            def proj_units(hb):
                """Output projection units for half-batch hb (y rows hb*128..+128)."""
                oall = proj_pool.tile(
                    [128, NCORES, 128], bf16, tag="oall", name=f"oall{hb}"
                )

                def load():
                    nc.sync.dma_start(
                        out=oall[:],
                        in_=recvs[hb][:].rearrange("i p t -> p i t"),
                    )

                def mm(n):
                    pso = ps_a.tile(
                        [128, 512], f32, tag="qkv", name=f"pso{hb}_{n}"
                    )
                    for i in range(NCC):
                        nc.tensor.matmul(
                            pso[:],
                            lhsT=oall[:, i, :],
                            rhs=pw_sb[:, i, n * 512:(n + 1) * 512],
                            start=(i == 0),
                            stop=(i == NCC - 1),
                        )
                    yt = y_pool.tile(
                        [128, 512], f32, tag="yt", name=f"yt{hb}_{n}"
                    )
                    nc.vector.tensor_add(
                        yt[:], pso[:], bias_sb[:, n * 512:(n + 1) * 512]
                    )
                    nc.sync.dma_start(
                        out=y_out[hb * 128:(hb + 1) * 128, n * 512:(n + 1) * 512],
                        in_=yt[:],
                    )

                return [lambda: (load(), mm(0)), lambda: mm(1)]

            def make_qkv(b):
                """Emit the xt DMA eagerly; return (tiles, PE work units) for
                batch b. Units are interleaved into the previous batch's
                attention rounds to keep the PE dense while the ACT engine
                works through the Exps."""
                QT = qt_pool.tile([128, T], bf16, tag="QT", name=f"QT{b}")
                KT = kt_pool.tile([128, T], bf16, tag="KT", name=f"KT{b}")
                VT = vt_pool.tile([128, T], bf16, tag="VT", name=f"VT{b}")
                VA = vaug_pool.tile([128, NKC, 256], bf16, tag="VA", name=f"VA{b}")
                nc.vector.memset(VA[:, :, 64:128], 1.0)
                nc.vector.memset(VA[:, :, 192:256], 1.0)
                dests = [QT, KT, VT]
                xt = xt_pool.tile([128, NCC, T], fp8, tag="xt", name=f"xt{b}")
                nc.sync.dma_start(out=xt[:], in_=xT[b])
                xt16s = []
                for half, eng in ((0, nc.scalar), (1, nc.gpsimd)):
                    xt16 = xt_pool.tile([128, NCC, 2 * TQ], bf16, tag="xt16",
                                        name=f"xt16_{b}_{half}")
                    eng.dma_start(out=xt16[:], in_=xT16[b, half])
                    xt16s.append(xt16)
                units = []

                def chain(t4, p3):
                    ps = ps_a.tile([128, TQ], f32, tag="qkv",
                                   name=f"qkv{b}_{t4}_{p3}")
                    if p3 < 2:
                        # q/k: fp8 DoubleRow (errors wash out in softmax)
                        for c2 in range(NCC // 2):
                            nc.tensor.matmul(
                                ps[:],
                                lhsT=w_sb[:, p3, 2 * c2:2 * c2 + 2, :],
                                rhs=xt[:, 2 * c2:2 * c2 + 2,
                                       t4 * TQ:(t4 + 1) * TQ],
                                start=(c2 == 0),
                                stop=(c2 == NCC // 2 - 1),
                                perf_mode=DR,
                            )
                        nc.vector.tensor_scalar_mul(
                            dests[p3][:, t4 * TQ:(t4 + 1) * TQ], ps[:],
                            1.0 / WSCALE,
                        )
                    else:
                        # v: bf16 (fp8 V noise lands directly on the output)
                        xsl = xt16s[t4 // 2][
                            :, :, (t4 % 2) * TQ:(t4 % 2 + 1) * TQ
                        ]
                        for i in range(NCC):
                            nc.tensor.matmul(
                                ps[:],
                                lhsT=wv_sb[:, i, :],
                                rhs=xsl[:, i, :],
                                start=(i == 0),
                                stop=(i == NCC - 1),
                            )
                        nc.vector.tensor_copy(
                            dests[p3][:, t4 * TQ:(t4 + 1) * TQ], ps[:]
                        )

                def transp(kc):
                    pst = ps_a.tile([128, 128], bf16, tag="qkv", name=f"pst{b}_{kc}")
                    nc.tensor.transpose(
                        pst[:], VT[:, kc * 128:(kc + 1) * 128], ident_sb[:]
                    )
                    out_ap = VA[:, kc, :].rearrange("p (g s) -> p g s", s=128)[
                        :, :, 0:64
                    ]
                    in_ap = pst[:].rearrange("p (g s) -> p g s", s=64)
                    nc.vector.tensor_copy(out_ap, in_ap)

                for t4 in range(NJ):
                    for p3 in range(3):
                        units.append(lambda t4=t4, p3=p3: chain(t4, p3))
                for kc2 in range(NKC // 2):
                    units.append(
                        lambda kc2=kc2: (transp(2 * kc2), transp(2 * kc2 + 1))
                    )
                return (QT, KT, VA), units

            cur, units = make_qkv(0)
            for u in units:
                u()
            load_proj_consts()

            pending_norm = []
            rnd = 0  # global attention round counter (never reset)

            def fire_a2a(hb):
                nc.gpsimd.collective_compute(
                    "AllToAll",
                    mybir.AluOpType.bypass,
                    replica_groups=[list(range(NCORES))],
                    ins=[sends[hb][:].opt()],
                    outs=[recvs[hb][:].opt()],
                )

            def score_exp(b, j, cpair, QT, KT):
                """Score matmuls (both heads concurrent via tile_position,
                diagonal chunks trimmed to the causal region), then Exp into
                fp8 E tiles + triangular mask / above-diagonal zero block."""
                pss2 = [
                    ps_s.tile([128, 2 * TQ], f32, tag="pss",
                              name=f"pss{b}_{j}_{cpair}_{h}")
                    for h in (0, 1)
                ]
                # h-inner order alternates PE row groups -> the two heads'
                # K=64 score matmuls run concurrently
                for ci in (0, 1):
                    c = 2 * cpair + ci
                    m = c - 4 * j
                    t0 = m * 128 if m > 0 else 0
                    for h in (0, 1):
                        nc.tensor.matmul(
                            pss2[h][:, ci * TQ + t0:(ci + 1) * TQ],
                            lhsT=KT[64 * h:64 * (h + 1), c * 128:(c + 1) * 128],
                            rhs=QT[64 * h:64 * (h + 1), j * TQ + t0:(j + 1) * TQ],
                            start=True,
                            stop=True,
                            tile_position=(64 * h, 0),
                        )
                m0 = 2 * cpair - 4 * j  # m of the pair's first chunk
                E2 = []
                for h in (0, 1):
                    E = e_pool.tile(
                        [128, 2 * TQ], bf16, tag="E",
                        name=f"E{b}_{j}_{cpair}_{h}",
                    )
                    if m0 >= 0:
                        # diagonal pair: exp the two trimmed halves
                        for ci in (0, 1):
                            m = m0 + ci
                            sl = slice(ci * TQ + m * 128, (ci + 1) * TQ)
                            nc.scalar.activation(
                                E[:, sl], pss2[h][:, sl], AF.Exp, scale=0.125
                            )
                    else:
                        nc.scalar.activation(E[:], pss2[h][:], AF.Exp, scale=0.125)
                    # triangular mask on diagonal chunks
                    for ci in (0, 1):
                        m = 2 * cpair + ci - 4 * j
                        if m >= 0:
                            sl = slice(ci * TQ + m * 128, ci * TQ + (m + 1) * 128)
                            nc.vector.tensor_mul(E[:, sl], E[:, sl], tri_sb[:])
                    E2.append(E)
                return E2

            for b in range(B):
                QT, KT, VA = cur
                if b + 1 < B:
                    cur, units = make_qkv(b + 1)
                else:
                    units = []
                # previous batch's projections, once their A2As have landed
                if b >= 1:
                    units = units + proj_units(2 * b - 2) + proj_units(2 * b - 1)
                ui = 0
                rb0 = rnd  # this batch's first round

                # attention for this batch, both heads, chunk-PAIR pipelined
                hb_done = {}
                for j in ((1, 0, 3, 2) if b + 1 == B else (3, 2, 1, 0)):
                    po = [
                        ps_o.tile([128, TQ], f32, tag=f"o{h}", name=f"po{h}_{b}_{j}")
                        for h in (0, 1)
                    ]
                    npairs = 2 * (j + 1)
                    av_queue = []  # exp'd pairs whose A'V is pending

                    def emit_av(item, j=j, po=po, VA=VA):
                        E2, cpair = item
                        for h in (0, 1):
                            for ci in (0, 1):
                                c = 2 * cpair + ci
                                m = c - 4 * j
                                cs = (
                                    slice(ci * TQ, (ci + 1) * TQ)
                                    if m < 0
                                    else slice(ci * TQ + m * 128, (ci + 1) * TQ)
                                )
                                ocs = slice(0, TQ) if m < 0 else slice(m * 128, TQ)
                                nc.tensor.matmul(
                                    po[h][:, ocs],
                                    lhsT=VA[:, c, 128 * h:128 * (h + 1)],
                                    rhs=E2[h][:, cs],
                                    start=(c == 0),
                                    stop=(c == 4 * j + 3),
                                )

                    for cpair in range(npairs):
                        av_queue.append(
                            (score_exp(b, j, cpair, QT, KT), cpair)
                        )
                        if len(av_queue) > 1:
                            emit_av(av_queue.pop(0))
                        nunit = 1 if (b + 1 < B or rnd - rb0 >= 14) else 0
                        if b + 1 < B and rnd - rb0 >= 12:
                            nunit += 1
                        for _ in range(nunit):
                            if ui < len(units):
                                units[ui]()
                                ui += 1
                        defer = 2 if b + 1 < B else 1
                        if pending_norm and rnd >= pending_norm[0][1] + defer:
                            pending_norm.pop(0)[0]()
                        rnd += 1
                    emit_av(av_queue.pop(0))

                    # copy O.T out of PSUM now (frees the accumulator bank);
                    # the rest of the normalize is deferred a round so the
                    # PE never waits on the DVE reciprocal
                    on_raws = []
                    for h in (0, 1):
                        on_raw = onorm_pool.tile(
                            [65, TQ], f32, tag="onr", name=f"onr{b}_{j}_{h}"
                        )
                        nc.vector.tensor_copy(on_raw[:], po[h][0:65, :])
                        on_raws.append(on_raw)

                    def norm_rest(b=b, j=j, on_raws=on_raws):
                        # the two heads' chains are interleaved per engine
                        # stage so their latencies overlap (matters in the
                        # exposed final norm before the last A2A)
                        hb = 2 * b + j // 2
                        rec4s, ris, rec1s, dens, ons = [], [], [], [], []
                        for h in (0, 1):
                            # reshape the denom row [1,512]->[4,128] so the
                            # reciprocal runs on 4 DVE lanes instead of 1
                            rec4 = small_pool.tile([4, 128], f32, tag="rec4")
                            nc.sync.dma_start(
                                out=rec4[:],
                                in_=on_raws[h][64:65, :]
                                .rearrange("p (q t) -> p q t", q=4),
                            )
                            rec4s.append(rec4)
                        for h in (0, 1):
                            ri = small_pool.tile([4, 128], f32, tag="ri")
                            nc.vector.reciprocal(ri[:], rec4s[h][:])
                            ris.append(ri)
                        for h in (0, 1):
                            rec1 = small_pool.tile([1, TQ], f32, tag="rec1")
                            nc.sync.dma_start(
                                out=rec1[:].rearrange("p (q t) -> p q t", q=4),
                                in_=ris[h][:],
                            )
                            rec1s.append(rec1)
                        for h in (0, 1):
                            den = small_pool.tile(
                                [64, TQ], f32, tag="den", name=f"den{b}_{j}_{h}"
                            )
                            nc.gpsimd.partition_broadcast(den[:], rec1s[h][:])
                            dens.append(den)
                        for h in (0, 1):
                            on = onorm_pool.tile(
                                [64, TQ], bf16, tag="on", name=f"on{b}_{j}_{h}"
                            )
                            nc.vector.tensor_mul(
                                on[:], on_raws[h][0:64, :], dens[h][:]
                            )
                            ons.append(on)
                        for h, eng in ((0, nc.sync), (1, nc.scalar)):
                            for q in range(4):
                                eng.dma_start(
                                    out=sends[hb][
                                        (j % 2) * 4 + q, 64 * h:64 * (h + 1), :
                                    ],
                                    in_=ons[h][:, q * 128:(q + 1) * 128],
                                )
                        hb_done[hb] = hb_done.get(hb, 0) + 1
                        if hb_done[hb] == 2:
                            fire_a2a(hb)

                    pending_norm.append((norm_rest, rnd))
                for u in units[ui:]:
                    u()
            for fn, _ in pending_norm:
                fn()
            pending_norm = []
            for u in proj_units(2 * B - 2) + proj_units(2 * B - 1):
                u()
    nc.finalize()
    return nc


_NC_CACHE = {}


def _get_nc():
    if "nc" not in _NC_CACHE:
        _NC_CACHE["nc"] = build_bass()
    return _NC_CACHE["nc"]


def _prep_inputs(x, Wk, Wq, Wv, proj_w, proj_b):
    x = np.asarray(x, dtype=np.float32)
    # [B, C, T] -> [B, 128, NCC, T] so the per-batch load is one contiguous
    # 16KB run per partition
    xp = x.transpose(0, 2, 1).reshape(B, NCC, 128, T).transpose(0, 2, 1, 3)
    xTs = np.ascontiguousarray(xp).astype(FP8)  # [B, 128, NCC, T]
    xT16s = np.ascontiguousarray(
        xp.reshape(B, 128, NCC, 2, 2 * TQ).transpose(0, 3, 1, 2, 4)
    ).astype(BF16)  # [B, 2, 128, NCC, 2*TQ]
    # pw[p, i, e] = proj_w.T[i*128+p, e]
    pw_r = np.ascontiguousarray(
        np.asarray(proj_w, np.float32).T.reshape(NCC, 128, C).transpose(1, 0, 2)
    ).astype(BF16)
    biasb = np.ascontiguousarray(
        np.broadcast_to(np.asarray(proj_b, np.float32), (128, C))
    )
    in_maps = []
    for core in range(NCORES):
        h0 = 2 * core

        def pack(W):
            W2 = np.concatenate(
                [np.asarray(W[h0], np.float32), np.asarray(W[h0 + 1], np.float32)],
                axis=1,
            )  # [C, 2D]
            return W2.reshape(NCC, 128, 2 * D)

        # w[p, p3, i, c2] = WSCALE * W[p3][i*128+p, c2]
        wq = np.stack([pack(Wq), pack(Wk)], axis=0)  # [2, NCC, 128, 2D]
        wq = np.ascontiguousarray(wq.transpose(2, 0, 1, 3)) * WSCALE
        wv = np.ascontiguousarray(pack(Wv).transpose(1, 0, 2)).astype(BF16)
        in_maps.append(
            {
                "xT": xTs,
                "xT16": xT16s,
                "w": wq.astype(FP8),
                "wv": wv,
                "pw": pw_r,
                "biasb": biasb,
            }
        )
    return in_maps


def _assemble(results):
    """Core r's y rows [hb*128, (hb+1)*128) = tokens
    b*2048 + (2*(hb%2) + r//4)*512 + (r%4)*128 + [0, 128), b = hb//2."""
    out = np.empty((B * T, C), np.float32)
    for r in range(NCORES):
        y = results[r]["y"]
        for hb in range(2 * B):
            b = hb // 2
            base = (2 * (hb % 2) + r // 4) * TQ + (r % 4) * 128
            out[b * T + base:b * T + base + 128] = y[hb * 128:(hb + 1) * 128]
    return out.reshape(B, T, C)


def kernel(x, Wk, Wq, Wv, proj_w, proj_b, _trace=False, _trace_kwargs=None):
    in_maps = _prep_inputs(x, Wk, Wq, Wv, proj_w, proj_b)
    nc = _get_nc()
    kw = {}
    if _trace:
        kw = dict(trace=True, trace_kwargs=_trace_kwargs or {})
    res = run_bass_kernel_spmd(nc, in_maps, core_ids=list(range(NCORES)), **kw)
    out = _assemble(res.results)
    if _trace:
        return out, res
    return out


if __name__ == "__main__":
    d = np.load("/root/problem/cache_io.npz")
    out = kernel(d["x"], d["Wk"], d["Wq"], d["Wv"], d["proj_w"], d["proj_b"])
    ref = d["ref"]
    err = np.abs(out - ref).max() / np.abs(ref).max()
    print("Relative error:", err)


# revision 23
# speedup vs baseline: 1.0462x; 1.0462x over previous
"""Multi-head causal attention (B=4, T=2048, 16 heads x 64) on 8 trn2 NeuronCores.

Sharding: tensor-parallel over heads (2 heads/core) for QKV projection +
attention; one AllToAll reshard per half-batch (head-sharded -> token-sharded),
fired as each half-batch finishes so only the last one is exposed; the output
projection for each half-batch's tokens is interleaved one batch behind.

The PE streams ~1 row/cycle at ~1.2 GHz regardless of operand dtype, so PE
time is cut by cutting ROWS: QKV projection and A'V run in fp8e4m3 DoubleRow
mode (two contraction slabs per instruction, 0.5 cycles/row); scores stay
bf16 (K=64, both heads packed via tile_position); the output projection is
bf16. fp8 quantization errors average out under attention. Weights are
host-scaled by 64 so fp8e4m3 sees mid-range values; the 1/64 is folded into
the PSUM->SBUF copies.

Per-core dataflow:
  - x.T host-packed to the exact SBUF layout [128, chunk, T] in fp8 (one
    16 KB descriptor per partition - the naive rearrange DMA was
    descriptor-generation-bound at ~19 us); one tile per batch.
  - QKV DoubleRow: lhsT = w[128, 2 slabs, 128], rhs = xt[:, 2c:2c+2, tq tile]
    -> Q.T, K.T [128=2*64 headdim, T] bf16 (x 1/64); V.T fp8 (x 1/64); V.T is
    PE-transposed into V_aug [ks, chunk, 256] fp8: per chunk
    [V_h0 | pad=1, V_h1 | pad=1] with col 64/192 the softmax-denominator
    ones column (cols 65..127 also 1.0: PSUM rows 65..127 are ignored;
    a 128-wide stationary streams 1 c/row where 65-wide streamed 1.5).
  - Scores transposed, S.T[ks,tq] = K @ Q.T, diagonal chunks trimmed to the
    causal region; ks-chunk PAIRS share one [128,1024] PSUM tile; Exp on the
    pair in one ACT op (diagonal pairs: two trimmed Exps + a zeroed
    above-diagonal block for the DoubleRow union range) -> E fp8; one
    128x128 triangular mask multiply per diagonal chunk. Softmax without
    max-subtraction (scores in [-0.6, 0.6]).
  - A'V DoubleRow per chunk-pair: rhs = E [128, 2, tq], lhsT = V_aug pair
    -> O.T|denom [128, tq] accumulated. Score matmuls for pair k+1 are
    emitted before A'V of pair k so the PE never waits on the Exp.
  - normalize: copy O.T out of PSUM (frees the bank); denom row reshaped
    [1,512]->[4,128] by SBUF DMA so the DVE reciprocal runs 4 lanes
    (0.85us vs 3.34); gpsimd partition_broadcast x4; DVE mul -> bf16.
  - output projection (bf16) over the 8 gathered head-chunks; bias add.

One AllToAll per half-batch (8 total; all but the last hidden under
compute), each moving bf16 [8 shards, 128 hd, 128 tok]: for half-batch
(b, jj), shard (j-2jj)*4+q holds tokens b*2048 + j*512 + q*128 + [0,128);
rank r owns (j = 2jj + r//4, q = r%4) of every half-batch = 8 x 128 rows.
"""

import numpy as np
import ml_dtypes

import concourse.bacc as bacc
import concourse.tile as tile
from concourse import mybir
from concourse.bass_utils import run_bass_kernel_spmd

NCORES = 8
B, T, C, H, D = 4, 2048, 1024, 16, 64
TQ = 512          # moving-dim tile for scores / A'V
NKC = T // 128    # ks 128-chunks per batch (16)
NJ = T // TQ      # tq tiles per batch (4)
NCC = C // 128    # contraction chunks for projections (8)
WSCALE = 64.0     # host-side qkv weight scale for fp8 range

f32 = mybir.dt.float32
bf16 = mybir.dt.bfloat16
fp8 = mybir.dt.float8e4
AF = mybir.ActivationFunctionType
DR = mybir.MatmulPerfMode.DoubleRow
BF16 = ml_dtypes.bfloat16
FP8 = ml_dtypes.float8_e4m3


def build_bass():
    nc = bacc.Bacc(None, num_devices=NCORES)

    # x.T pre-packed to SBUF layout per 512-token tile: [b][t4][p][chunk][t]
    xT = nc.dram_tensor("xT", [B, NJ, 128, NCC, TQ], fp8, kind="ExternalInput")
    # bf16 x.T for the V chain, same tiling
    xT16 = nc.dram_tensor("xT16", [B, NJ, 128, NCC, TQ], bf16,
                          kind="ExternalInput")
    # per-core q,k weights (x WSCALE), host-prearranged: [row, 2(q,k), chunk, 2D]
    w_in = nc.dram_tensor("w", [128, 2, NCC, 2 * D], fp8, kind="ExternalInput")
    # per-core v weights, bf16 unscaled
    wv_in = nc.dram_tensor("wv", [128, NCC, 2 * D], bf16, kind="ExternalInput")
    # proj_w.T host-prearranged: [row-in-chunk, chunk, C]
    pw_in = nc.dram_tensor("pw", [128, NCC, C], bf16, kind="ExternalInput")
    bias_in = nc.dram_tensor("biasb", [128, C], f32, kind="ExternalInput")
    y_out = nc.dram_tensor("y", [B * T // NCORES, C], f32, kind="ExternalOutput")

    ident_np = np.eye(128, dtype=np.float32).astype(BF16)
    tri_np = (np.arange(128)[None, :] >= np.arange(128)[:, None]).astype(BF16)
    ident_dram = nc.inline_tensor(ident_np, name="ident")
    tri_dram = nc.inline_tensor(tri_np, name="trimask")

    with tile.TileContext(nc, num_cores=NCORES) as tc, nc.allow_low_precision(
        reason="fp8/bf16 kernel; tolerance 2e-2"
    ):
        with (
            tc.tile_pool(name="dram", bufs=1, space="DRAM") as dpool,
            tc.tile_pool(name="consts", bufs=1) as consts,
            tc.tile_pool(name="xt", bufs=5) as xt_pool,
            tc.tile_pool(name="qt", bufs=2) as qt_pool,
            tc.tile_pool(name="kt", bufs=2) as kt_pool,
            tc.tile_pool(name="vt", bufs=1) as vt_pool,
            tc.tile_pool(name="vaug", bufs=2) as vaug_pool,
            tc.tile_pool(name="e", bufs=4) as e_pool,
            tc.tile_pool(name="onorm", bufs=4) as onorm_pool,
            tc.tile_pool(name="small", bufs=2) as small_pool,
            tc.tile_pool(name="proj", bufs=2) as proj_pool,
            tc.tile_pool(name="ytile", bufs=2) as y_pool,
            tc.tile_pool(name="ps_a", bufs=2, space="PSUM") as ps_a,
            tc.tile_pool(name="ps_s", bufs=2, space="PSUM") as ps_s,
            tc.tile_pool(name="ps_o", bufs=1, space="PSUM") as ps_o,
        ):
            sends = [
                dpool.tile([NCORES, 2 * D, 128], bf16, tag="send", name=f"send{hb}")
                for hb in range(2 * B)
            ]
            recvs = [
                dpool.tile([NCORES, 2 * D, 128], bf16, tag="recv", name=f"recv{hb}")
                for hb in range(2 * B)
            ]

            w_sb = consts.tile([128, 2, NCC, 2 * D], fp8)
            nc.gpsimd.dma_start(out=w_sb[:], in_=w_in[:])
            wv_sb = consts.tile([128, NCC, 2 * D], bf16)
            nc.gpsimd.dma_start(out=wv_sb[:], in_=wv_in[:])
            ident_sb = consts.tile([128, 128], bf16)
            nc.gpsimd.dma_start(out=ident_sb[:], in_=ident_dram[:])
            tri_sb = consts.tile([128, 128], bf16)
            nc.gpsimd.dma_start(out=tri_sb[:], in_=tri_dram[:])
            pw_sb = consts.tile([128, NCC, C], bf16)
            bias_sb = consts.tile([128, C], f32)

            def load_proj_consts():
                nc.gpsimd.dma_start(out=pw_sb[:], in_=pw_in[:])
                nc.gpsimd.dma_start(out=bias_sb[:], in_=bias_in[:])

            def proj_units(hb):
                """Output projection units for half-batch hb (y rows hb*128..+128)."""
                oall = proj_pool.tile(
                    [128, NCORES, 128], bf16, tag="oall", name=f"oall{hb}"
                )

                def load():
                    nc.sync.dma_start(
                        out=oall[:],
                        in_=recvs[hb][:].rearrange("i p t -> p i t"),
                    )

                def mm(n):
                    pso = ps_a.tile(
                        [128, 512], f32, tag="qkv", name=f"pso{hb}_{n}"
                    )
                    for i in range(NCC):
                        nc.tensor.matmul(
                            pso[:],
                            lhsT=oall[:, i, :],
                            rhs=pw_sb[:, i, n * 512:(n + 1) * 512],
                            start=(i == 0),
                            stop=(i == NCC - 1),
                        )
                    yt = y_pool.tile(
                        [128, 512], f32, tag="yt", name=f"yt{hb}_{n}"
                    )
                    nc.vector.tensor_add(
                        yt[:], pso[:], bias_sb[:, n * 512:(n + 1) * 512]
                    )
                    nc.sync.dma_start(
                        out=y_out[hb * 128:(hb + 1) * 128, n * 512:(n + 1) * 512],
                        in_=yt[:],
                    )

                return [lambda: (load(), mm(0)), lambda: mm(1)]

            def make_qkv(b):
                """Emit the xt DMA eagerly; return (tiles, PE work units) for
                batch b. Units are interleaved into the previous batch's
                attention rounds to keep the PE dense while the ACT engine
                works through the Exps."""
                QT = qt_pool.tile([128, T], bf16, tag="QT", name=f"QT{b}")
                KT = kt_pool.tile([128, T], bf16, tag="KT", name=f"KT{b}")
                VT = vt_pool.tile([128, T], bf16, tag="VT", name=f"VT{b}")
                VA = vaug_pool.tile([128, NKC, 256], bf16, tag="VA", name=f"VA{b}")
                nc.vector.memset(VA[:, :, 64:128], 1.0)
                nc.vector.memset(VA[:, :, 192:256], 1.0)
                dests = [QT, KT, VT]
                xts, xt16s = [], []
                for t4 in range(NJ):
                    xt = xt_pool.tile([128, NCC, TQ], fp8, tag="xt",
                                      name=f"xt{b}_{t4}")
                    nc.sync.dma_start(out=xt[:], in_=xT[b, t4])
                    xts.append(xt)
                for t4 in range(NJ):
                    xt16 = xt_pool.tile([128, NCC, TQ], bf16, tag="xt16",
                                        name=f"xt16_{b}_{t4}")
                    (nc.scalar if t4 % 2 == 0 else nc.gpsimd).dma_start(
                        out=xt16[:], in_=xT16[b, t4]
                    )
                    xt16s.append(xt16)
                units = []

                def chain(t4, p3):
                    ps = ps_a.tile([128, TQ], f32, tag="qkv",
                                   name=f"qkv{b}_{t4}_{p3}")
                    if p3 < 2:
                        # q/k: fp8 DoubleRow (errors wash out in softmax)
                        for c2 in range(NCC // 2):
                            nc.tensor.matmul(
                                ps[:],
                                lhsT=w_sb[:, p3, 2 * c2:2 * c2 + 2, :],
                                rhs=xts[t4][:, 2 * c2:2 * c2 + 2, :],
                                start=(c2 == 0),
                                stop=(c2 == NCC // 2 - 1),
                                perf_mode=DR,
                            )
                        nc.vector.tensor_scalar_mul(
                            dests[p3][:, t4 * TQ:(t4 + 1) * TQ], ps[:],
                            1.0 / WSCALE,
                        )
                    else:
                        # v: bf16 (fp8 V noise lands directly on the output)
                        xsl = xt16s[t4][:, :, :]
                        for i in range(NCC):
                            nc.tensor.matmul(
                                ps[:],
                                lhsT=wv_sb[:, i, :],
                                rhs=xsl[:, i, :],
                                start=(i == 0),
                                stop=(i == NCC - 1),
                            )
                        nc.vector.tensor_copy(
                            dests[p3][:, t4 * TQ:(t4 + 1) * TQ], ps[:]
                        )

                def transp(kc):
                    pst = ps_a.tile([128, 128], bf16, tag="qkv", name=f"pst{b}_{kc}")
                    nc.tensor.transpose(
                        pst[:], VT[:, kc * 128:(kc + 1) * 128], ident_sb[:]
                    )
                    out_ap = VA[:, kc, :].rearrange("p (g s) -> p g s", s=128)[
                        :, :, 0:64
                    ]
                    in_ap = pst[:].rearrange("p (g s) -> p g s", s=64)
                    nc.vector.tensor_copy(out_ap, in_ap)

                for t4 in range(NJ):
                    for p3 in range(3):
                        units.append(lambda t4=t4, p3=p3: chain(t4, p3))
                for kc2 in range(NKC // 2):
                    units.append(
                        lambda kc2=kc2: (transp(2 * kc2), transp(2 * kc2 + 1))
                    )
                return (QT, KT, VA), units

            cur, units = make_qkv(0)
            for u in units:
                u()
            load_proj_consts()

            pending_norm = []
            rnd = 0  # global attention round counter (never reset)

            def fire_a2a(hb):
                nc.gpsimd.collective_compute(
                    "AllToAll",
                    mybir.AluOpType.bypass,
                    replica_groups=[list(range(NCORES))],
                    ins=[sends[hb][:].opt()],
                    outs=[recvs[hb][:].opt()],
                )

            def score_exp(b, j, cpair, QT, KT):
                """Score matmuls (both heads concurrent via tile_position,
                diagonal chunks trimmed to the causal region), then Exp into
                fp8 E tiles + triangular mask / above-diagonal zero block."""
                pss2 = [
                    ps_s.tile([128, 2 * TQ], f32, tag="pss",
                              name=f"pss{b}_{j}_{cpair}_{h}")
                    for h in (0, 1)
                ]
                # h-inner order alternates PE row groups -> the two heads'
                # K=64 score matmuls run concurrently
                for ci in (0, 1):
                    c = 2 * cpair + ci
                    m = c - 4 * j
                    t0 = m * 128 if m > 0 else 0
                    for h in (0, 1):
                        nc.tensor.matmul(
                            pss2[h][:, ci * TQ + t0:(ci + 1) * TQ],
                            lhsT=KT[64 * h:64 * (h + 1), c * 128:(c + 1) * 128],
                            rhs=QT[64 * h:64 * (h + 1), j * TQ + t0:(j + 1) * TQ],
                            start=True,
                            stop=True,
                            tile_position=(64 * h, 0),
                        )
                m0 = 2 * cpair - 4 * j  # m of the pair's first chunk
                E2 = []
                for h in (0, 1):
                    E = e_pool.tile(
                        [128, 2 * TQ], bf16, tag="E",
                        name=f"E{b}_{j}_{cpair}_{h}",
                    )
                    if m0 >= 0:
                        # diagonal pair: exp the two trimmed halves
                        for ci in (0, 1):
                            m = m0 + ci
                            sl = slice(ci * TQ + m * 128, (ci + 1) * TQ)
                            nc.scalar.activation(
                                E[:, sl], pss2[h][:, sl], AF.Exp, scale=0.125
                            )
                    else:
                        nc.scalar.activation(E[:], pss2[h][:], AF.Exp, scale=0.125)
                    # triangular mask on diagonal chunks
                    for ci in (0, 1):
                        m = 2 * cpair + ci - 4 * j
                        if m >= 0:
                            sl = slice(ci * TQ + m * 128, ci * TQ + (m + 1) * 128)
                            nc.vector.tensor_mul(E[:, sl], E[:, sl], tri_sb[:])
                    E2.append(E)
                return E2

            for b in range(B):
                QT, KT, VA = cur
                if b + 1 < B:
                    cur, units = make_qkv(b + 1)
                else:
                    units = []
                # previous batch's projections, once their A2As have landed
                if b >= 1:
                    units = units + proj_units(2 * b - 2) + proj_units(2 * b - 1)
                ui = 0
                rb0 = rnd  # this batch's first round

                # attention for this batch, both heads, chunk-PAIR pipelined
                hb_done = {}
                for j in ((1, 0, 3, 2) if b + 1 == B else (3, 2, 1, 0)):
                    po = [
                        ps_o.tile([128, TQ], f32, tag=f"o{h}", name=f"po{h}_{b}_{j}")
                        for h in (0, 1)
                    ]
                    npairs = 2 * (j + 1)
                    av_queue = []  # exp'd pairs whose A'V is pending

                    def emit_av(item, j=j, po=po, VA=VA):
                        E2, cpair = item
                        for h in (0, 1):
                            for ci in (0, 1):
                                c = 2 * cpair + ci
                                m = c - 4 * j
                                cs = (
                                    slice(ci * TQ, (ci + 1) * TQ)
                                    if m < 0
                                    else slice(ci * TQ + m * 128, (ci + 1) * TQ)
                                )
                                ocs = slice(0, TQ) if m < 0 else slice(m * 128, TQ)
                                nc.tensor.matmul(
                                    po[h][:, ocs],
                                    lhsT=VA[:, c, 128 * h:128 * (h + 1)],
                                    rhs=E2[h][:, cs],
                                    start=(c == 0),
                                    stop=(c == 4 * j + 3),
                                )

                    for cpair in range(npairs):
                        av_queue.append(
                            (score_exp(b, j, cpair, QT, KT), cpair)
                        )
                        if len(av_queue) > 1:
                            emit_av(av_queue.pop(0))
                        nunit = 1 if (b + 1 < B or rnd - rb0 >= 14) else 0
                        if b + 1 < B and rnd - rb0 >= 12:
                            nunit += 1
                        for _ in range(nunit):
                            if ui < len(units):
                                units[ui]()
                                ui += 1
                        defer = 2 if b + 1 < B else 1
                        if pending_norm and rnd >= pending_norm[0][1] + defer:
                            pending_norm.pop(0)[0]()
                        rnd += 1
                    emit_av(av_queue.pop(0))

                    # copy O.T out of PSUM now (frees the accumulator bank);
                    # the rest of the normalize is deferred a round so the
                    # PE never waits on the DVE reciprocal
                    on_raws = []
                    for h in (0, 1):
                        on_raw = onorm_pool.tile(
                            [65, TQ], f32, tag="onr", name=f"onr{b}_{j}_{h}"
                        )
                        nc.vector.tensor_copy(on_raw[:], po[h][0:65, :])
                        on_raws.append(on_raw)

                    def norm_rest(b=b, j=j, on_raws=on_raws):
                        # the two heads' chains are interleaved per engine
                        # stage so their latencies overlap (matters in the
                        # exposed final norm before the last A2A)
                        hb = 2 * b + j // 2
                        rec4s, ris, rec1s, dens, ons = [], [], [], [], []
                        for h in (0, 1):
                            # reshape the denom row [1,512]->[4,128] so the
                            # reciprocal runs on 4 DVE lanes instead of 1
                            rec4 = small_pool.tile([4, 128], f32, tag="rec4")
                            nc.sync.dma_start(
                                out=rec4[:],
                                in_=on_raws[h][64:65, :]
                                .rearrange("p (q t) -> p q t", q=4),
                            )
                            rec4s.append(rec4)
                        for h in (0, 1):
                            ri = small_pool.tile([4, 128], f32, tag="ri")
                            nc.vector.reciprocal(ri[:], rec4s[h][:])
                            ris.append(ri)
                        for h in (0, 1):
                            rec1 = small_pool.tile([1, TQ], f32, tag="rec1")
                            nc.sync.dma_start(
                                out=rec1[:].rearrange("p (q t) -> p q t", q=4),
                                in_=ris[h][:],
                            )
                            rec1s.append(rec1)
                        for h in (0, 1):
                            den = small_pool.tile(
                                [64, TQ], f32, tag="den", name=f"den{b}_{j}_{h}"
                            )
                            nc.gpsimd.partition_broadcast(den[:], rec1s[h][:])
                            dens.append(den)
                        for h in (0, 1):
                            on = onorm_pool.tile(
                                [64, TQ], bf16, tag="on", name=f"on{b}_{j}_{h}"
                            )
                            nc.vector.tensor_mul(
                                on[:], on_raws[h][0:64, :], dens[h][:]
                            )
                            ons.append(on)
                        for h, eng in ((0, nc.sync), (1, nc.scalar)):
                            for q in range(4):
                                eng.dma_start(
                                    out=sends[hb][
                                        (j % 2) * 4 + q, 64 * h:64 * (h + 1), :
                                    ],
                                    in_=ons[h][:, q * 128:(q + 1) * 128],
                                )
                        hb_done[hb] = hb_done.get(hb, 0) + 1
                        if hb_done[hb] == 2:
                            fire_a2a(hb)

                    pending_norm.append((norm_rest, rnd))
                for u in units[ui:]:
                    u()
            for fn, _ in pending_norm:
                fn()
            pending_norm = []
            for u in proj_units(2 * B - 2) + proj_units(2 * B - 1):
                u()
    nc.finalize()
    return nc


_NC_CACHE = {}


def _get_nc():
    if "nc" not in _NC_CACHE:
        _NC_CACHE["nc"] = build_bass()
    return _NC_CACHE["nc"]


def _prep_inputs(x, Wk, Wq, Wv, proj_w, proj_b):
    x = np.asarray(x, dtype=np.float32)
    # [B, C, T] -> [B, 128, NCC, T] so the per-batch load is one contiguous
    # 16KB run per partition
    xp = (
        x.transpose(0, 2, 1)
        .reshape(B, NCC, 128, NJ, TQ)
        .transpose(0, 3, 2, 1, 4)
    )  # [B, t4, p, chunk, t]
    xTs = np.ascontiguousarray(xp).astype(FP8)
    xT16s = np.ascontiguousarray(xp).astype(BF16)
    # pw[p, i, e] = proj_w.T[i*128+p, e]
    pw_r = np.ascontiguousarray(
        np.asarray(proj_w, np.float32).T.reshape(NCC, 128, C).transpose(1, 0, 2)
    ).astype(BF16)
    biasb = np.ascontiguousarray(
        np.broadcast_to(np.asarray(proj_b, np.float32), (128, C))
    )
    in_maps = []
    for core in range(NCORES):
        h0 = 2 * core

        def pack(W):
            W2 = np.concatenate(
                [np.asarray(W[h0], np.float32), np.asarray(W[h0 + 1], np.float32)],
                axis=1,
            )  # [C, 2D]
            return W2.reshape(NCC, 128, 2 * D)

        # w[p, p3, i, c2] = WSCALE * W[p3][i*128+p, c2]
        wq = np.stack([pack(Wq), pack(Wk)], axis=0)  # [2, NCC, 128, 2D]
        wq = np.ascontiguousarray(wq.transpose(2, 0, 1, 3)) * WSCALE
        wv = np.ascontiguousarray(pack(Wv).transpose(1, 0, 2)).astype(BF16)
        in_maps.append(
            {
                "xT": xTs,
                "xT16": xT16s,
                "w": wq.astype(FP8),
                "wv": wv,
                "pw": pw_r,
                "biasb": biasb,
            }
        )
    return in_maps


def _assemble(results):
    """Core r's y rows [hb*128, (hb+1)*128) = tokens
    b*2048 + (2*(hb%2) + r//4)*512 + (r%4)*128 + [0, 128), b = hb//2."""
    out = np.empty((B * T, C), np.float32)
    for r in range(NCORES):
        y = results[r]["y"]
        for hb in range(2 * B):
            b = hb // 2
            base = (2 * (hb % 2) + r // 4) * TQ + (r % 4) * 128
            out[b * T + base:b * T + base + 128] = y[hb * 128:(hb + 1) * 128]
    return out.reshape(B, T, C)


def kernel(x, Wk, Wq, Wv, proj_w, proj_b, _trace=False, _trace_kwargs=None):
    in_maps = _prep_inputs(x, Wk, Wq, Wv, proj_w, proj_b)
    nc = _get_nc()
    kw = {}
    if _trace:
        kw = dict(trace=True, trace_kwargs=_trace_kwargs or {})
    res = run_bass_kernel_spmd(nc, in_maps, core_ids=list(range(NCORES)), **kw)
    out = _assemble(res.results)
    if _trace:
        return out, res
    return out


if __name__ == "__main__":
    d = np.load("/root/problem/cache_io.npz")
    out = kernel(d["x"], d["Wk"], d["Wq"], d["Wv"], d["proj_w"], d["proj_b"])
    ref = d["ref"]
    err = np.abs(out - ref).max() / np.abs(ref).max()
    print("Relative error:", err)
